# revision 21
# baseline (speedup 1.0000x reference)
"""Trainium2 Bass kernel for the real-space Ewald potential.

Computes  out = NORM/(4*pi) * sum_{i!=j} (q_i . q_j) * erf(|r_i-r_j|/sqrt(2)) / |r_i-r_j|

Strategy (8 NeuronCores, SPMD):
  - The N x N pair grid is split into 8x8 super-tiles of 512x512; core c
    processes row c of the grid, rotated so the diagonal super-tile is the
    core-local tile 0 (identical program, per-core data).
  - d2_ij = s_i + s_j - 2 r_i.r_j comes from ONE K=18 bf16 matmul: r and s
    are split hi/lo (hi/mid/lo for s) into bf16 on the host, so the PE runs
    at full bf16 rate while keeping |d2 err| < 2e-4 (an fp32 matmul runs
    2 passes and is ~2-4x slower).
  - d = sqrt(d2 + 5e-4) and erf(d/sqrt(2)) on the scalar (ACT) engine in
    two strictly separated phases (sqrt and erf live in different ACT
    table sets; interleaving would reload tables at ~2.7us each time).
    1/d via the single-instruction DVE reciprocal_approx_fast in phase 1.
  - F = erf(d/sqrt(2)) * (1/d) multiplies are split between the vector and
    GPSIMD engines (bf16 output); the diagonal is zeroed via a precomputed
    mask on the diagonal super-tile.
  - G[c,i] = sum_j q[j,c] F[j,i] is a K=128 bf16 matmul with q ALSO split
    hi/lo (lhsT [qh|ql], M=16) so q's bf16 rounding cancels; four
    super-tiles share a PSUM bank via PE column-tile quadrants 0/32/64/96.
    The final contraction sum_i q[i,c] G[c,i] is a multiply+reduce on the
    vector engine plus a ones-vector matmul. Each core emits one scalar
    partial; the host sums the 8 partials and applies the constant scale.
"""

import os
import sys

import ml_dtypes
import numpy as np

for _p in ("/opt/trn_rl_repo",):
    if os.path.isdir(_p) and _p not in sys.path:
        sys.path.insert(0, _p)

import concourse.bacc as bacc  # noqa: E402
import concourse.mybir as mybir  # noqa: E402
import concourse.tile as tile  # noqa: E402
from concourse.bass_utils import run_bass_kernel_spmd  # noqa: E402

N = 4096  # atoms
NQ = 8  # charge channels
NCORES = 8
CH = 512  # super-tile edge (i-chunk width / j-chunk height)
NU = 9  # half-super-tile units per core: 72 halves / 8 cores, exact balance
NGB = (NU + 3) // 4  # G PSUM banks (4 units per bank, PE quadrants)
BIAS = 5e-4  # sqrt(d2 + BIAS): guards bf16-split cancellation (|err| < 2e-4)
INV_SQRT2 = 0.7071067811865476
TWOPI = 2.0 * np.pi
NORM_FACTOR = 90.0474
BF16 = ml_dtypes.bfloat16

# Quadratic-minimax reciprocal constants: with nx = bitcast(~x), t = x*nx
# lands in [-4.5, -4] for any positive fp32 x; 1/x ~ nx*(RA + t*(RB + RC*t))
# to 5.1e-5 relative. Fused with the erf multiply into ONE custom DVE op.
RECIP_A = -0.707106429
RECIP_B = -0.166521999
RECIP_C = -0.013060550


def _register_emul_recip():
    """Register the fused f = in1 * (1/in0) custom DVE op (8 ALU stages)."""
    import concourse.dve_ops as dve_ops
    from concourse.dve_spec import (
        C0,
        C1,
        C2,
        AluOp,
        Bin,
        Spec,
        Src0,
        Src1,
        _has_src1,
        lower as _dve_lower,
    )
    from concourse.dve_uop import DveOpSpec

    name = "EMUL_RECIP_Q_ANT"
    for op in dve_ops.OPS:
        if op.name == name:
            return op

    _nx = Bin(AluOp.BITWISE_NOT, Src0, Src0)
    _t = Src0 * _nx

    def _ref(in0, in1, c0, c1, c2):
        nx = (~np.asarray(in0, np.float32).view(np.int32)).view(np.float32)
        t = in0 * nx
        return ((c0 + t * (c1 + c2 * t)) * nx) * in1

    spec = Spec(body=((C0 + _t * (C1 + C2 * _t)) * _nx) * Src1, reference=_ref)
    row = max(dve_ops._SUB_OPCODE_FOR_NAME.values()) + 1
    assert row < 0x20
    dve_ops._SUB_OPCODE_FOR_NAME[name] = row
    shas = {}
    for ver in ("v3", "v4"):
        s = DveOpSpec(
            name=name, opcode=row, uops=_dve_lower(spec, ver=ver), rd1_en=_has_src1(spec)
        )
        shas[ver] = s.sha(ver)
    op = dve_ops.DveOp(name, spec, subdim=False, uops_sha=shas)
    dve_ops.OPS.append(op)
    dve_ops.CUSTOM_DVE_SPECS[name] = spec
    return op


EMUL_RECIP_Q = _register_emul_recip()

TRACE = bool(os.environ.get("BASS_EWALD_TRACE"))
LAST_RESULTS = None  # BassKernelResults of the most recent run (for test.py)

_prog = None


def _finalize_bank(nc, sp, gk, qf, acc, k):
    OP = mybir.AluOpType
    f32 = mybir.dt.float32
    prod = sp.tile([128, CH], f32, tag=f"prod{k}")
    nc.vector.tensor_tensor(
        prod[:], gk[:], qf[:, k * CH : (k + 1) * CH], OP.mult
    )
    nc.vector.reduce_sum(acc[:, k : k + 1], prod[:], axis=mybir.AxisListType.X)


def _build_program():
    f32 = mybir.dt.float32
    bf16 = mybir.dt.bfloat16
    AF = mybir.ActivationFunctionType
    OP = mybir.AluOpType

    nc = bacc.Bacc("TRN2", target_bir_lowering=False, debug=False, num_devices=NCORES)
    at_d = nc.dram_tensor("AT", [18, NU * CH], bf16, kind="ExternalInput")
    bt_d = nc.dram_tensor("BT", [18, NU * 256], bf16, kind="ExternalInput")
    qw_d = nc.dram_tensor("QW", [128, NU * 32], bf16, kind="ExternalInput")
    qf_d = nc.dram_tensor("QF", [128, NGB * CH], f32, kind="ExternalInput")
    out_d = nc.dram_tensor("OUT", [1, 1], f32, kind="ExternalOutput")

    with tile.TileContext(nc) as tc:
        with (
            tc.tile_pool(name="const", bufs=1) as cp,
            tc.tile_pool(name="work", bufs=3) as wp,
            tc.tile_pool(name="single", bufs=1) as sp,
            tc.tile_pool(name="pd", bufs=2, space="PSUM") as pd,
            tc.tile_pool(name="pg", bufs=1, space="PSUM") as pg,
        ):
            at = cp.tile([18, NU * CH], bf16)
            bt = cp.tile([18, NU * 256], bf16)
            # chunked loads on two DMA queues: unit 0's operands land first
            # so the first d2 matmul isn't gated on the whole load.
            # a tiny first chunk so unit 0's matmul starts as soon as possible
            for lo, hi in ((0, 1), (1, 5), (5, NU)):
                nc.sync.dma_start(at[:, lo * CH : hi * CH], at_d[:, lo * CH : hi * CH])
                nc.gpsimd.dma_start(
                    bt[:, lo * 256 : hi * 256], bt_d[:, lo * 256 : hi * 256]
                )
            qw = cp.tile([128, NU * 32], bf16)
            nc.gpsimd.dma_start(qw[:], qw_d[:])
            qf = cp.tile([128, NGB * CH], f32)
            dall = cp.tile([128, NU * 1024], f32)
            ones = cp.tile([128, 1], f32)
            nc.vector.memset(ones[:], 1.0)
            bias_t = cp.tile([128, 1], f32)
            nc.vector.memset(bias_t[:], BIAS)
            gbanks = []
            for k in range(NGB):
                gk = pg.tile([128, CH], f32, tag=f"g{k}")
                nc.vector.memset(gk[:], 0.0)
                gbanks.append(gk)

            # Phase 1: d2 matmuls + sqrt (sqrt ACT table set).
            for u in range(NU):
                ps = pd.tile([128, 1024], f32, tag="d2")
                for loc in (0, 1):
                    nc.tensor.matmul(
                        ps[:, loc * CH : (loc + 1) * CH],
                        bt[:, u * 256 + loc * 128 : u * 256 + (loc + 1) * 128],
                        at[:, u * CH : (u + 1) * CH],
                        start=True,
                        stop=True,
                    )
                dsl = dall[:, u * 1024 : (u + 1) * 1024]
                nc.scalar.activation(dsl, ps[:], AF.Sqrt, bias=bias_t[:])

            # qf is only needed by the finalize stage; issuing its DMA after
            # phase 1 keeps the head of the sync queue clear for AT/BT.
            nc.gpsimd.dma_start(qf[:], qf_d[:])

            # Keep the two ACT table sets in disjoint program ranges.
            tc.no_sync_barrier()

            # Phase 2: erf (paired units, FD=2048) + F-multiply + G matmuls.
            acc = sp.tile([128, NGB], f32, tag="acc")
            ets = {}
            for u in range(NU):
                k, m = divmod(u, 4)  # G bank, quadrant
                if u % 2 == 0:
                    span = min(2, NU - u)
                    et = wp.tile([128, span * 1024], f32, tag="e")
                    nc.scalar.activation(
                        et[:],
                        dall[:, u * 1024 : (u + span) * 1024],
                        AF.Erf,
                        scale=INV_SQRT2,
                    )
                    ets[u] = et
                    esl = et[:, 0:1024]
                else:
                    esl = ets[u - 1][:, 1024:2048]
                dsl = dall[:, u * 1024 : (u + 1) * 1024]
                f = wp.tile([128, 1024], bf16, tag="f")
                nc.vector._custom_dve(
                    EMUL_RECIP_Q,
                    out=f[:],
                    in0=dsl,
                    in1=esl,
                    s0=RECIP_A,
                    s1=RECIP_B,
                    imm2=RECIP_C,
                )
                for loc in (0, 1):
                    nc.tensor.matmul(
                        gbanks[k][32 * m : 32 * m + 16, :],
                        qw[:, u * 32 + loc * 16 : u * 32 + (loc + 1) * 16],
                        f[:, loc * CH : (loc + 1) * CH],
                        start=(loc == 0),
                        stop=(loc == 1),
                        tile_position=(0, 32 * m),
                    )
                if u == 3:
                    _finalize_bank(nc, sp, gbanks[0], qf, acc, 0)
                elif u == 7:
                    _finalize_bank(nc, sp, gbanks[1], qf, acc, 1)

            _finalize_bank(nc, sp, gbanks[2], qf, acc, 2)
            accsum = sp.tile([128, 1], f32, tag="accsum")
            nc.vector.reduce_sum(accsum[:], acc[:], axis=mybir.AxisListType.X)
            tot = pg.tile([1, 1], f32, tag="tot")
            nc.tensor.matmul(tot[:], accsum[:], ones[:], start=True, stop=True)
            res = sp.tile([1, 1], f32, tag="res")
            nc.scalar.copy(res[:], tot[:])
            nc.sync.dma_start(out_d[:], res[:])

    nc.compile()
    return nc


def _get_program():
    global _prog
    if _prog is None:
        _prog = _build_program()
    return _prog


def _bf16_split(x32, parts):
    """Split fp32 array into `parts` bf16 arrays summing to x32 (greedy)."""
    out = []
    rem = x32.astype(np.float64)
    for _ in range(parts):
        p = rem.astype(np.float32).astype(BF16)
        out.append(p)
        rem = rem - p.astype(np.float64)
    return out


def _host_prep(q, r):
    q = np.ascontiguousarray(np.asarray(q, np.float32))
    r = np.ascontiguousarray(np.asarray(r, np.float32))
    r64 = r.astype(np.float64)
    s64 = (r64 * r64).sum(1)

    rh, rl = _bf16_split(r, 2)  # [N,3] bf16 each
    m2rh, m2rl = (-2.0 * rh.astype(np.float32)).astype(BF16), (
        -2.0 * rl.astype(np.float32)
    ).astype(BF16)
    sh, sm, sl = _bf16_split(s64, 3)  # [N] bf16 each
    onesN = np.ones(N, BF16)

    # rhs rows (i side) pair with lhsT rows (j side), K=18:
    #   -2rh_j*rh_i, -2rh_j*rl_i, -2rl_j*rh_i, -2rl_j*rl_i (12 rows),
    #   (sh+sm+sl)_j * 1 (3 rows), 1 * (sh+sm+sl)_i (3 rows)
    A18 = np.concatenate(
        [rh.T, rl.T, rh.T, rl.T, [onesN, onesN, onesN], [sh, sm, sl]]
    ).astype(BF16)  # [18, N]
    B18 = np.concatenate(
        [m2rh.T, m2rh.T, m2rl.T, m2rl.T, [sh, sm, sl], [onesN, onesN, onesN]]
    ).astype(BF16)  # [18, N]

    qT = np.ascontiguousarray(q.T)  # [NQ, N] f32

    # 72 half-super-tiles of the symmetric pair grid (8 diagonal pairs w=1 +
    # 28 upper-triangle pairs w=2, each split into j-block halves hh=0/1),
    # dealt round-robin: exactly 9 units per core, no dummy work.
    pairs = [(c, c, 1.0) for c in range(8)] + [
        (a, b, 2.0) for a in range(8) for b in range(a + 1, 8)
    ]
    units = [(a, b, hh, w) for (a, b, w) in pairs for hh in (0, 1)]
    assignments = [[] for _ in range(NCORES)]
    for idx, unit in enumerate(units):
        assignments[idx % NCORES].append(unit)

    in_maps = []
    for c in range(NCORES):
        AT = np.empty((18, NU * CH), BF16)
        BT = np.empty((18, NU * 256), BF16)
        QW = np.empty((128, NU * 32), BF16)
        QF = np.zeros((128, NGB * CH), np.float32)
        for u, (a, b, hh, w) in enumerate(assignments[c]):
            k, m = divmod(u, 4)
            AT[:, u * CH : (u + 1) * CH] = A18[:, b * CH : (b + 1) * CH]
            BT[:, u * 256 : (u + 1) * 256] = B18[
                :, a * CH + hh * 256 : a * CH + (hh + 1) * 256
            ]
            # Finalize reads quadrant rows 32m + [0..16): both the qh and ql
            # halves of G contract against the same fp32 qT chunk.
            QF[32 * m : 32 * m + NQ, k * CH : (k + 1) * CH] = qT[
                :, b * CH : (b + 1) * CH
            ]
            QF[32 * m + NQ : 32 * m + 2 * NQ, k * CH : (k + 1) * CH] = qT[
                :, b * CH : (b + 1) * CH
            ]
            wq = (
                w * q[a * CH + hh * 256 : a * CH + (hh + 1) * 256, :]
            ).astype(np.float32)  # [256, NQ]
            wqh, wql = _bf16_split(wq, 2)
            blk = np.concatenate([wqh, wql], axis=1)  # [256, 16]
            QW[:, u * 32 : (u + 1) * 32] = (
                blk.reshape(2, 128, 2 * NQ).transpose(1, 0, 2).reshape(128, 32)
            )
        in_maps.append({"AT": AT, "BT": BT, "QW": QW, "QF": QF})
    return in_maps


def _diag_constant():
    """F value the device computes on the (unmasked) pair-grid diagonal.

    d2 on the diagonal is |err| < 2e-4, and F(x) = erf(sqrt((x+B)/2)) /
    sqrt(x+B) is flat there (variation < 1e-4 relative), so every diagonal
    element lands on the same bf16 value: bf16 applied to the device's
    erf * quadratic-reciprocal product at d = sqrt(BIAS). The bf16 bucket
    is 0.4% wide -- vastly wider than the variation -- so this is exact."""
    from scipy.special import erf as _erf

    d0 = np.float32(np.sqrt(BIAS))
    e0 = np.float32(_erf(float(d0) * INV_SQRT2))
    nx = (~d0.reshape(1).view(np.int32)).view(np.float32)[0]
    t = np.float32(d0 * nx)
    rq = np.float32(
        (np.float32(RECIP_A) + t * (np.float32(RECIP_B) + np.float32(RECIP_C) * t))
        * nx
    )
    return float((e0 * rq).astype(BF16))


def kernel(q, r, cell):
    global LAST_RESULTS
    in_maps = _host_prep(q, r)
    nc = _get_program()
    res = run_bass_kernel_spmd(nc, in_maps, list(range(NCORES)), trace=TRACE)
    LAST_RESULTS = res
    S = sum(float(res.results[c]["OUT"][0, 0]) for c in range(NCORES))
    S -= _diag_constant() * float((q.astype(np.float64) ** 2).sum())
    val = S / TWOPI / 2.0 * NORM_FACTOR
    return np.array([val], np.float32)


# revision 22
# speedup vs baseline: 1.0046x; 1.0046x over previous
"""Trainium2 Bass kernel for the real-space Ewald potential.

Computes  out = NORM/(4*pi) * sum_{i!=j} (q_i . q_j) * erf(|r_i-r_j|/sqrt(2)) / |r_i-r_j|

Strategy (8 NeuronCores, SPMD):
  - The N x N pair grid is split into 8x8 super-tiles of 512x512; core c
    processes row c of the grid, rotated so the diagonal super-tile is the
    core-local tile 0 (identical program, per-core data).
  - d2_ij = s_i + s_j - 2 r_i.r_j comes from ONE K=18 bf16 matmul: r and s
    are split hi/lo (hi/mid/lo for s) into bf16 on the host, so the PE runs
    at full bf16 rate while keeping |d2 err| < 2e-4 (an fp32 matmul runs
    2 passes and is ~2-4x slower).
  - d = sqrt(d2 + 5e-4) and erf(d/sqrt(2)) on the scalar (ACT) engine in
    two strictly separated phases (sqrt and erf live in different ACT
    table sets; interleaving would reload tables at ~2.7us each time).
    1/d via the single-instruction DVE reciprocal_approx_fast in phase 1.
  - F = erf(d/sqrt(2)) * (1/d) multiplies are split between the vector and
    GPSIMD engines (bf16 output); the diagonal is zeroed via a precomputed
    mask on the diagonal super-tile.
  - G[c,i] = sum_j q[j,c] F[j,i] is a K=128 bf16 matmul with q ALSO split
    hi/lo (lhsT [qh|ql], M=16) so q's bf16 rounding cancels; four
    super-tiles share a PSUM bank via PE column-tile quadrants 0/32/64/96.
    The final contraction sum_i q[i,c] G[c,i] is a multiply+reduce on the
    vector engine plus a ones-vector matmul. Each core emits one scalar
    partial; the host sums the 8 partials and applies the constant scale.
"""

import os
import sys

import ml_dtypes
import numpy as np

for _p in ("/opt/trn_rl_repo",):
    if os.path.isdir(_p) and _p not in sys.path:
        sys.path.insert(0, _p)

import concourse.bacc as bacc  # noqa: E402
import concourse.mybir as mybir  # noqa: E402
import concourse.tile as tile  # noqa: E402
from concourse.bass_utils import run_bass_kernel_spmd  # noqa: E402

N = 4096  # atoms
NQ = 8  # charge channels
NCORES = 8
CH = 512  # super-tile edge (i-chunk width / j-chunk height)
NU = 9  # half-super-tile units per core: 72 halves / 8 cores, exact balance
NGB = (NU + 3) // 4  # G PSUM banks (4 units per bank, PE quadrants)
BIAS = 5e-4  # sqrt(d2 + BIAS): guards bf16-split cancellation (|err| < 2e-4)
INV_SQRT2 = 0.7071067811865476
TWOPI = 2.0 * np.pi
NORM_FACTOR = 90.0474
BF16 = ml_dtypes.bfloat16

# Quadratic-minimax reciprocal constants: with nx = bitcast(~x), t = x*nx
# lands in [-4.5, -4] for any positive fp32 x; 1/x ~ nx*(RA + t*(RB + RC*t))
# to 5.1e-5 relative. Fused with the erf multiply into ONE custom DVE op.
RECIP_A = -0.707106429
RECIP_B = -0.166521999
RECIP_C = -0.013060550


def _register_emul_recip():
    """Register the fused f = in1 * (1/in0) custom DVE op (8 ALU stages)."""
    import concourse.dve_ops as dve_ops
    from concourse.dve_spec import (
        C0,
        C1,
        C2,
        AluOp,
        Bin,
        Spec,
        Src0,
        Src1,
        _has_src1,
        lower as _dve_lower,
    )
    from concourse.dve_uop import DveOpSpec

    name = "EMUL_RECIP_Q_ANT"
    for op in dve_ops.OPS:
        if op.name == name:
            return op

    _nx = Bin(AluOp.BITWISE_NOT, Src0, Src0)
    _t = Src0 * _nx

    def _ref(in0, in1, c0, c1, c2):
        nx = (~np.asarray(in0, np.float32).view(np.int32)).view(np.float32)
        t = in0 * nx
        return ((c0 + t * (c1 + c2 * t)) * nx) * in1

    spec = Spec(body=((C0 + _t * (C1 + C2 * _t)) * _nx) * Src1, reference=_ref)
    row = max(dve_ops._SUB_OPCODE_FOR_NAME.values()) + 1
    assert row < 0x20
    dve_ops._SUB_OPCODE_FOR_NAME[name] = row
    shas = {}
    for ver in ("v3", "v4"):
        s = DveOpSpec(
            name=name, opcode=row, uops=_dve_lower(spec, ver=ver), rd1_en=_has_src1(spec)
        )
        shas[ver] = s.sha(ver)
    op = dve_ops.DveOp(name, spec, subdim=False, uops_sha=shas)
    dve_ops.OPS.append(op)
    dve_ops.CUSTOM_DVE_SPECS[name] = spec
    return op


EMUL_RECIP_Q = _register_emul_recip()

TRACE = bool(os.environ.get("BASS_EWALD_TRACE"))
LAST_RESULTS = None  # BassKernelResults of the most recent run (for test.py)

_prog = None


def _finalize_bank(nc, sp, gk, qf, acc, k):
    OP = mybir.AluOpType
    f32 = mybir.dt.float32
    prod = sp.tile([128, CH], f32, tag=f"prod{k}")
    nc.vector.tensor_tensor(
        prod[:], gk[:], qf[:, k * CH : (k + 1) * CH], OP.mult
    )
    nc.vector.reduce_sum(acc[:, k : k + 1], prod[:], axis=mybir.AxisListType.X)


def _build_program():
    f32 = mybir.dt.float32
    bf16 = mybir.dt.bfloat16
    AF = mybir.ActivationFunctionType
    OP = mybir.AluOpType

    nc = bacc.Bacc("TRN2", target_bir_lowering=False, debug=False, num_devices=NCORES)
    at_d = nc.dram_tensor("AT", [18, NU * CH], bf16, kind="ExternalInput")
    bt_d = nc.dram_tensor("BT", [18, NU * 256], bf16, kind="ExternalInput")
    qw_d = nc.dram_tensor("QW", [128, NU * 32], bf16, kind="ExternalInput")
    qf_d = nc.dram_tensor("QF", [128, NGB * CH], f32, kind="ExternalInput")
    out_d = nc.dram_tensor("OUT", [1, 1], f32, kind="ExternalOutput")

    with tile.TileContext(nc) as tc:
        with (
            tc.tile_pool(name="const", bufs=1) as cp,
            tc.tile_pool(name="work", bufs=3) as wp,
            tc.tile_pool(name="single", bufs=1) as sp,
            tc.tile_pool(name="pd", bufs=2, space="PSUM") as pd,
            tc.tile_pool(name="pg", bufs=1, space="PSUM") as pg,
        ):
            at = cp.tile([18, NU * CH], bf16)
            bt = cp.tile([18, NU * 256], bf16)
            # chunked loads on two DMA queues: unit 0's operands land first
            # so the first d2 matmul isn't gated on the whole load.
            for p in range(3):
                sl = slice(p * 3 * CH, min((p + 1) * 3, NU) * CH)
                nc.sync.dma_start(at[:, sl], at_d[:, sl])
                sl2 = slice(p * 3 * 256, min((p + 1) * 3, NU) * 256)
                nc.gpsimd.dma_start(bt[:, sl2], bt_d[:, sl2])
            qw = cp.tile([128, NU * 32], bf16)
            nc.gpsimd.dma_start(qw[:], qw_d[:])
            qf = cp.tile([128, NGB * CH], f32)
            dall = cp.tile([128, NU * 1024], f32)
            ones = cp.tile([128, 1], f32)
            nc.vector.memset(ones[:], 1.0)
            bias_t = cp.tile([128, 1], f32)
            nc.vector.memset(bias_t[:], BIAS)
            gbanks = []
            for k in range(NGB):
                gk = pg.tile([128, CH], f32, tag=f"g{k}")
                nc.vector.memset(gk[:], 0.0)
                gbanks.append(gk)

            # Phase 1: d2 matmuls + sqrt (sqrt ACT table set).
            for u in range(NU):
                ps = pd.tile([128, 1024], f32, tag="d2")
                for loc in (0, 1):
                    nc.tensor.matmul(
                        ps[:, loc * CH : (loc + 1) * CH],
                        bt[:, u * 256 + loc * 128 : u * 256 + (loc + 1) * 128],
                        at[:, u * CH : (u + 1) * CH],
                        start=True,
                        stop=True,
                    )
                dsl = dall[:, u * 1024 : (u + 1) * 1024]
                nc.scalar.activation(dsl, ps[:], AF.Sqrt, bias=bias_t[:])

            # qf is only needed by the finalize stage; issuing its DMA after
            # phase 1 keeps the head of the sync queue clear for AT/BT.
            nc.gpsimd.dma_start(qf[:], qf_d[:])

            # Keep the two ACT table sets in disjoint program ranges.
            tc.no_sync_barrier()

            # Phase 2: erf (paired units, FD=2048) + F-multiply + G matmuls.
            acc = sp.tile([128, NGB], f32, tag="acc")
            ets = {}
            for u in range(NU):
                k, m = divmod(u, 4)  # G bank, quadrant
                if u % 2 == 0:
                    span = min(2, NU - u)
                    et = wp.tile([128, span * 1024], f32, tag="e")
                    nc.scalar.activation(
                        et[:],
                        dall[:, u * 1024 : (u + span) * 1024],
                        AF.Erf,
                        scale=INV_SQRT2,
                    )
                    ets[u] = et
                    esl = et[:, 0:1024]
                else:
                    esl = ets[u - 1][:, 1024:2048]
                dsl = dall[:, u * 1024 : (u + 1) * 1024]
                f = wp.tile([128, 1024], bf16, tag="f")
                nc.vector._custom_dve(
                    EMUL_RECIP_Q,
                    out=f[:],
                    in0=dsl,
                    in1=esl,
                    s0=RECIP_A,
                    s1=RECIP_B,
                    imm2=RECIP_C,
                )
                for loc in (0, 1):
                    nc.tensor.matmul(
                        gbanks[k][32 * m : 32 * m + 16, :],
                        qw[:, u * 32 + loc * 16 : u * 32 + (loc + 1) * 16],
                        f[:, loc * CH : (loc + 1) * CH],
                        start=(loc == 0),
                        stop=(loc == 1),
                        tile_position=(0, 32 * m),
                    )
                if u == 3:
                    _finalize_bank(nc, sp, gbanks[0], qf, acc, 0)
                elif u == 7:
                    _finalize_bank(nc, sp, gbanks[1], qf, acc, 1)

            _finalize_bank(nc, sp, gbanks[2], qf, acc, 2)
            accsum = sp.tile([128, 1], f32, tag="accsum")
            nc.vector.reduce_sum(accsum[:], acc[:], axis=mybir.AxisListType.X)
            tot = pg.tile([1, 1], f32, tag="tot")
            nc.tensor.matmul(tot[:], accsum[:], ones[:], start=True, stop=True)
            res = sp.tile([1, 1], f32, tag="res")
            nc.scalar.copy(res[:], tot[:])
            nc.sync.dma_start(out_d[:], res[:])

    nc.compile()
    return nc


def _get_program():
    global _prog
    if _prog is None:
        _prog = _build_program()
    return _prog


def _bf16_split(x32, parts):
    """Split fp32 array into `parts` bf16 arrays summing to x32 (greedy)."""
    out = []
    rem = x32.astype(np.float64)
    for _ in range(parts):
        p = rem.astype(np.float32).astype(BF16)
        out.append(p)
        rem = rem - p.astype(np.float64)
    return out


def _host_prep(q, r):
    q = np.ascontiguousarray(np.asarray(q, np.float32))
    r = np.ascontiguousarray(np.asarray(r, np.float32))
    r64 = r.astype(np.float64)
    s64 = (r64 * r64).sum(1)

    rh, rl = _bf16_split(r, 2)  # [N,3] bf16 each
    m2rh, m2rl = (-2.0 * rh.astype(np.float32)).astype(BF16), (
        -2.0 * rl.astype(np.float32)
    ).astype(BF16)
    sh, sm, sl = _bf16_split(s64, 3)  # [N] bf16 each
    onesN = np.ones(N, BF16)

    # rhs rows (i side) pair with lhsT rows (j side), K=18:
    #   -2rh_j*rh_i, -2rh_j*rl_i, -2rl_j*rh_i, -2rl_j*rl_i (12 rows),
    #   (sh+sm+sl)_j * 1 (3 rows), 1 * (sh+sm+sl)_i (3 rows)
    A18 = np.concatenate(
        [rh.T, rl.T, rh.T, rl.T, [onesN, onesN, onesN], [sh, sm, sl]]
    ).astype(BF16)  # [18, N]
    B18 = np.concatenate(
        [m2rh.T, m2rh.T, m2rl.T, m2rl.T, [sh, sm, sl], [onesN, onesN, onesN]]
    ).astype(BF16)  # [18, N]

    qT = np.ascontiguousarray(q.T)  # [NQ, N] f32

    # 72 half-super-tiles of the symmetric pair grid (8 diagonal pairs w=1 +
    # 28 upper-triangle pairs w=2, each split into j-block halves hh=0/1),
    # dealt round-robin: exactly 9 units per core, no dummy work.
    pairs = [(c, c, 1.0) for c in range(8)] + [
        (a, b, 2.0) for a in range(8) for b in range(a + 1, 8)
    ]
    units = [(a, b, hh, w) for (a, b, w) in pairs for hh in (0, 1)]
    assignments = [[] for _ in range(NCORES)]
    for idx, unit in enumerate(units):
        assignments[idx % NCORES].append(unit)

    in_maps = []
    for c in range(NCORES):
        AT = np.empty((18, NU * CH), BF16)
        BT = np.empty((18, NU * 256), BF16)
        QW = np.empty((128, NU * 32), BF16)
        QF = np.zeros((128, NGB * CH), np.float32)
        for u, (a, b, hh, w) in enumerate(assignments[c]):
            k, m = divmod(u, 4)
            AT[:, u * CH : (u + 1) * CH] = A18[:, b * CH : (b + 1) * CH]
            BT[:, u * 256 : (u + 1) * 256] = B18[
                :, a * CH + hh * 256 : a * CH + (hh + 1) * 256
            ]
            # Finalize reads quadrant rows 32m + [0..16): both the qh and ql
            # halves of G contract against the same fp32 qT chunk.
            QF[32 * m : 32 * m + NQ, k * CH : (k + 1) * CH] = qT[
                :, b * CH : (b + 1) * CH
            ]
            QF[32 * m + NQ : 32 * m + 2 * NQ, k * CH : (k + 1) * CH] = qT[
                :, b * CH : (b + 1) * CH
            ]
            wq = (
                w * q[a * CH + hh * 256 : a * CH + (hh + 1) * 256, :]
            ).astype(np.float32)  # [256, NQ]
            wqh, wql = _bf16_split(wq, 2)
            blk = np.concatenate([wqh, wql], axis=1)  # [256, 16]
            QW[:, u * 32 : (u + 1) * 32] = (
                blk.reshape(2, 128, 2 * NQ).transpose(1, 0, 2).reshape(128, 32)
            )
        in_maps.append({"AT": AT, "BT": BT, "QW": QW, "QF": QF})
    return in_maps


def _diag_constant():
    """F value the device computes on the (unmasked) pair-grid diagonal.

    d2 on the diagonal is |err| < 2e-4, and F(x) = erf(sqrt((x+B)/2)) /
    sqrt(x+B) is flat there (variation < 1e-4 relative), so every diagonal
    element lands on the same bf16 value: bf16 applied to the device's
    erf * quadratic-reciprocal product at d = sqrt(BIAS). The bf16 bucket
    is 0.4% wide -- vastly wider than the variation -- so this is exact."""
    from scipy.special import erf as _erf

    d0 = np.float32(np.sqrt(BIAS))
    e0 = np.float32(_erf(float(d0) * INV_SQRT2))
    nx = (~d0.reshape(1).view(np.int32)).view(np.float32)[0]
    t = np.float32(d0 * nx)
    rq = np.float32(
        (np.float32(RECIP_A) + t * (np.float32(RECIP_B) + np.float32(RECIP_C) * t))
        * nx
    )
    return float((e0 * rq).astype(BF16))


def kernel(q, r, cell):
    global LAST_RESULTS
    in_maps = _host_prep(q, r)
    nc = _get_program()
    res = run_bass_kernel_spmd(nc, in_maps, list(range(NCORES)), trace=TRACE)
    LAST_RESULTS = res
    S = sum(float(res.results[c]["OUT"][0, 0]) for c in range(NCORES))
    S -= _diag_constant() * float((q.astype(np.float64) ** 2).sum())
    val = S / TWOPI / 2.0 * NORM_FACTOR
    return np.array([val], np.float32)


# revision 23
# speedup vs baseline: 1.0196x; 1.0149x over previous
"""Trainium2 Bass kernel for the real-space Ewald potential.

Computes  out = NORM/(4*pi) * sum_{i!=j} (q_i . q_j) * erf(|r_i-r_j|/sqrt(2)) / |r_i-r_j|

Strategy (8 NeuronCores, SPMD):
  - The N x N pair grid is split into 8x8 super-tiles of 512x512; core c
    processes row c of the grid, rotated so the diagonal super-tile is the
    core-local tile 0 (identical program, per-core data).
  - d2_ij = s_i + s_j - 2 r_i.r_j comes from ONE K=18 bf16 matmul: r and s
    are split hi/lo (hi/mid/lo for s) into bf16 on the host, so the PE runs
    at full bf16 rate while keeping |d2 err| < 2e-4 (an fp32 matmul runs
    2 passes and is ~2-4x slower).
  - d = sqrt(d2 + 5e-4) and erf(d/sqrt(2)) on the scalar (ACT) engine in
    two strictly separated phases (sqrt and erf live in different ACT
    table sets; interleaving would reload tables at ~2.7us each time).
    1/d via the single-instruction DVE reciprocal_approx_fast in phase 1.
  - F = erf(d/sqrt(2)) * (1/d) multiplies are split between the vector and
    GPSIMD engines (bf16 output); the diagonal is zeroed via a precomputed
    mask on the diagonal super-tile.
  - G[c,i] = sum_j q[j,c] F[j,i] is a K=128 bf16 matmul with q ALSO split
    hi/lo (lhsT [qh|ql], M=16) so q's bf16 rounding cancels; four
    super-tiles share a PSUM bank via PE column-tile quadrants 0/32/64/96.
    The final contraction sum_i q[i,c] G[c,i] is a multiply+reduce on the
    vector engine plus a ones-vector matmul. Each core emits one scalar
    partial; the host sums the 8 partials and applies the constant scale.
"""

import os
import sys

import ml_dtypes
import numpy as np

for _p in ("/opt/trn_rl_repo",):
    if os.path.isdir(_p) and _p not in sys.path:
        sys.path.insert(0, _p)

import concourse.bacc as bacc  # noqa: E402
import concourse.mybir as mybir  # noqa: E402
import concourse.tile as tile  # noqa: E402
from concourse.bass_utils import run_bass_kernel_spmd  # noqa: E402

N = 4096  # atoms
NQ = 8  # charge channels
NCORES = 8
CH = 512  # super-tile edge (i-chunk width / j-chunk height)
NU = 9  # half-super-tile units per core: 72 halves / 8 cores, exact balance
NGB = (NU + 3) // 4  # G PSUM banks (4 units per bank, PE quadrants)
BIAS = 5e-4  # sqrt(d2 + BIAS): guards bf16-split cancellation (|err| < 2e-4)
INV_SQRT2 = 0.7071067811865476
TWOPI = 2.0 * np.pi
NORM_FACTOR = 90.0474
BF16 = ml_dtypes.bfloat16

# Quadratic-minimax reciprocal constants: with nx = bitcast(~x), t = x*nx
# lands in [-4.5, -4] for any positive fp32 x; 1/x ~ nx*(RA + t*(RB + RC*t))
# to 5.1e-5 relative. Fused with the erf multiply into ONE custom DVE op.
RECIP_A = -0.707106429
RECIP_B = -0.166521999
RECIP_C = -0.013060550


def _register_emul_recip():
    """Register the fused f = in1 * (1/in0) custom DVE op (8 ALU stages)."""
    import concourse.dve_ops as dve_ops
    from concourse.dve_spec import (
        C0,
        C1,
        C2,
        AluOp,
        Bin,
        Spec,
        Src0,
        Src1,
        _has_src1,
        lower as _dve_lower,
    )
    from concourse.dve_uop import DveOpSpec

    name = "EMUL_RECIP_Q_ANT"
    for op in dve_ops.OPS:
        if op.name == name:
            return op

    _nx = Bin(AluOp.BITWISE_NOT, Src0, Src0)
    _t = Src0 * _nx

    def _ref(in0, in1, c0, c1, c2):
        nx = (~np.asarray(in0, np.float32).view(np.int32)).view(np.float32)
        t = in0 * nx
        return ((c0 + t * (c1 + c2 * t)) * nx) * in1

    spec = Spec(body=((C0 + _t * (C1 + C2 * _t)) * _nx) * Src1, reference=_ref)
    row = max(dve_ops._SUB_OPCODE_FOR_NAME.values()) + 1
    assert row < 0x20
    dve_ops._SUB_OPCODE_FOR_NAME[name] = row
    shas = {}
    for ver in ("v3", "v4"):
        s = DveOpSpec(
            name=name, opcode=row, uops=_dve_lower(spec, ver=ver), rd1_en=_has_src1(spec)
        )
        shas[ver] = s.sha(ver)
    op = dve_ops.DveOp(name, spec, subdim=False, uops_sha=shas)
    dve_ops.OPS.append(op)
    dve_ops.CUSTOM_DVE_SPECS[name] = spec
    return op


EMUL_RECIP_Q = _register_emul_recip()

TRACE = bool(os.environ.get("BASS_EWALD_TRACE"))
LAST_RESULTS = None  # BassKernelResults of the most recent run (for test.py)

_prog = None


def _finalize_bank(nc, sp, gk, qf, acc, k):
    OP = mybir.AluOpType
    f32 = mybir.dt.float32
    prod = sp.tile([128, CH], f32, tag=f"prod{k}")
    nc.vector.tensor_tensor(
        prod[:], gk[:], qf[:, k * CH : (k + 1) * CH], OP.mult
    )
    nc.vector.reduce_sum(acc[:, k : k + 1], prod[:], axis=mybir.AxisListType.X)


def _build_program():
    f32 = mybir.dt.float32
    bf16 = mybir.dt.bfloat16
    AF = mybir.ActivationFunctionType
    OP = mybir.AluOpType

    nc = bacc.Bacc("TRN2", target_bir_lowering=False, debug=False, num_devices=NCORES)
    at_d = nc.dram_tensor("AT", [18, NU * CH], bf16, kind="ExternalInput")
    bt_d = nc.dram_tensor("BT", [18, NU * 256], bf16, kind="ExternalInput")
    qw_d = nc.dram_tensor("QW", [128, NU * 32], bf16, kind="ExternalInput")
    qf_d = nc.dram_tensor("QF", [128, NGB * CH], f32, kind="ExternalInput")
    out_d = nc.dram_tensor("OUT", [1, 1], f32, kind="ExternalOutput")

    with tile.TileContext(nc) as tc:
        with (
            tc.tile_pool(name="const", bufs=1) as cp,
            tc.tile_pool(name="work", bufs=4) as wp,
            tc.tile_pool(name="single", bufs=1) as sp,
            tc.tile_pool(name="pd", bufs=2, space="PSUM") as pd,
            tc.tile_pool(name="pg", bufs=1, space="PSUM") as pg,
        ):
            at = cp.tile([18, NU * CH], bf16)
            bt = cp.tile([18, NU * 256], bf16)
            # chunked loads on two DMA queues: unit 0's operands land first
            # so the first d2 matmul isn't gated on the whole load.
            for p in range(3):
                sl = slice(p * 3 * CH, min((p + 1) * 3, NU) * CH)
                nc.sync.dma_start(at[:, sl], at_d[:, sl])
                sl2 = slice(p * 3 * 256, min((p + 1) * 3, NU) * 256)
                nc.gpsimd.dma_start(bt[:, sl2], bt_d[:, sl2])
            qw = cp.tile([128, NU * 32], bf16)
            nc.gpsimd.dma_start(qw[:], qw_d[:])
            qf = cp.tile([128, NGB * CH], f32)
            dall = cp.tile([128, NU * 1024], f32)
            ones = cp.tile([128, 1], f32)
            nc.vector.memset(ones[:], 1.0)
            bias_t = cp.tile([128, 1], f32)
            nc.vector.memset(bias_t[:], BIAS)
            gbanks = []
            for k in range(NGB):
                gk = pg.tile([128, CH], f32, tag=f"g{k}")
                nc.vector.memset(gk[:], 0.0)
                gbanks.append(gk)

            # Phase 1: d2 matmuls + sqrt (sqrt ACT table set).
            for u in range(NU):
                ps = pd.tile([128, 1024], f32, tag="d2")
                for loc in (0, 1):
                    nc.tensor.matmul(
                        ps[:, loc * CH : (loc + 1) * CH],
                        bt[:, u * 256 + loc * 128 : u * 256 + (loc + 1) * 128],
                        at[:, u * CH : (u + 1) * CH],
                        start=True,
                        stop=True,
                    )
                dsl = dall[:, u * 1024 : (u + 1) * 1024]
                nc.scalar.activation(dsl, ps[:], AF.Sqrt, bias=bias_t[:])

            # qf is only needed by the finalize stage; issuing its DMA after
            # phase 1 keeps the head of the sync queue clear for AT/BT.
            nc.gpsimd.dma_start(qf[:], qf_d[:])

            # Keep the two ACT table sets in disjoint program ranges.
            tc.no_sync_barrier()

            # Phase 2: erf (paired units, FD=2048) + F-multiply + G matmuls.
            acc = sp.tile([128, NGB], f32, tag="acc")
            ets = {}
            for u in range(NU):
                k, m = divmod(u, 4)  # G bank, quadrant
                if u % 2 == 0:
                    span = min(2, NU - u)
                    et = wp.tile([128, span * 1024], f32, tag="e")
                    nc.scalar.activation(
                        et[:],
                        dall[:, u * 1024 : (u + span) * 1024],
                        AF.Erf,
                        scale=INV_SQRT2,
                    )
                    ets[u] = et
                    esl = et[:, 0:1024]
                else:
                    esl = ets[u - 1][:, 1024:2048]
                dsl = dall[:, u * 1024 : (u + 1) * 1024]
                f = wp.tile([128, 1024], bf16, tag="f")
                nc.vector._custom_dve(
                    EMUL_RECIP_Q,
                    out=f[:],
                    in0=dsl,
                    in1=esl,
                    s0=RECIP_A,
                    s1=RECIP_B,
                    imm2=RECIP_C,
                )
                for loc in (0, 1):
                    nc.tensor.matmul(
                        gbanks[k][32 * m : 32 * m + 16, :],
                        qw[:, u * 32 + loc * 16 : u * 32 + (loc + 1) * 16],
                        f[:, loc * CH : (loc + 1) * CH],
                        start=(loc == 0),
                        stop=(loc == 1),
                        tile_position=(0, 32 * m),
                    )
                if u == 3:
                    _finalize_bank(nc, sp, gbanks[0], qf, acc, 0)
                elif u == 7:
                    _finalize_bank(nc, sp, gbanks[1], qf, acc, 1)

            _finalize_bank(nc, sp, gbanks[2], qf, acc, 2)
            accsum = sp.tile([128, 1], f32, tag="accsum")
            nc.vector.reduce_sum(accsum[:], acc[:], axis=mybir.AxisListType.X)
            tot = pg.tile([1, 1], f32, tag="tot")
            nc.tensor.matmul(tot[:], accsum[:], ones[:], start=True, stop=True)
            res = sp.tile([1, 1], f32, tag="res")
            nc.scalar.copy(res[:], tot[:])
            nc.sync.dma_start(out_d[:], res[:])

    nc.compile()
    return nc


def _get_program():
    global _prog
    if _prog is None:
        _prog = _build_program()
    return _prog


def _bf16_split(x32, parts):
    """Split fp32 array into `parts` bf16 arrays summing to x32 (greedy)."""
    out = []
    rem = x32.astype(np.float64)
    for _ in range(parts):
        p = rem.astype(np.float32).astype(BF16)
        out.append(p)
        rem = rem - p.astype(np.float64)
    return out


def _host_prep(q, r):
    q = np.ascontiguousarray(np.asarray(q, np.float32))
    r = np.ascontiguousarray(np.asarray(r, np.float32))
    r64 = r.astype(np.float64)
    s64 = (r64 * r64).sum(1)

    rh, rl = _bf16_split(r, 2)  # [N,3] bf16 each
    m2rh, m2rl = (-2.0 * rh.astype(np.float32)).astype(BF16), (
        -2.0 * rl.astype(np.float32)
    ).astype(BF16)
    sh, sm, sl = _bf16_split(s64, 3)  # [N] bf16 each
    onesN = np.ones(N, BF16)

    # rhs rows (i side) pair with lhsT rows (j side), K=18:
    #   -2rh_j*rh_i, -2rh_j*rl_i, -2rl_j*rh_i, -2rl_j*rl_i (12 rows),
    #   (sh+sm+sl)_j * 1 (3 rows), 1 * (sh+sm+sl)_i (3 rows)
    A18 = np.concatenate(
        [rh.T, rl.T, rh.T, rl.T, [onesN, onesN, onesN], [sh, sm, sl]]
    ).astype(BF16)  # [18, N]
    B18 = np.concatenate(
        [m2rh.T, m2rh.T, m2rl.T, m2rl.T, [sh, sm, sl], [onesN, onesN, onesN]]
    ).astype(BF16)  # [18, N]

    qT = np.ascontiguousarray(q.T)  # [NQ, N] f32

    # 72 half-super-tiles of the symmetric pair grid (8 diagonal pairs w=1 +
    # 28 upper-triangle pairs w=2, each split into j-block halves hh=0/1),
    # dealt round-robin: exactly 9 units per core, no dummy work.
    pairs = [(c, c, 1.0) for c in range(8)] + [
        (a, b, 2.0) for a in range(8) for b in range(a + 1, 8)
    ]
    units = [(a, b, hh, w) for (a, b, w) in pairs for hh in (0, 1)]
    assignments = [[] for _ in range(NCORES)]
    for idx, unit in enumerate(units):
        assignments[idx % NCORES].append(unit)

    in_maps = []
    for c in range(NCORES):
        AT = np.empty((18, NU * CH), BF16)
        BT = np.empty((18, NU * 256), BF16)
        QW = np.empty((128, NU * 32), BF16)
        QF = np.zeros((128, NGB * CH), np.float32)
        for u, (a, b, hh, w) in enumerate(assignments[c]):
            k, m = divmod(u, 4)
            AT[:, u * CH : (u + 1) * CH] = A18[:, b * CH : (b + 1) * CH]
            BT[:, u * 256 : (u + 1) * 256] = B18[
                :, a * CH + hh * 256 : a * CH + (hh + 1) * 256
            ]
            # Finalize reads quadrant rows 32m + [0..16): both the qh and ql
            # halves of G contract against the same fp32 qT chunk.
            QF[32 * m : 32 * m + NQ, k * CH : (k + 1) * CH] = qT[
                :, b * CH : (b + 1) * CH
            ]
            QF[32 * m + NQ : 32 * m + 2 * NQ, k * CH : (k + 1) * CH] = qT[
                :, b * CH : (b + 1) * CH
            ]
            wq = (
                w * q[a * CH + hh * 256 : a * CH + (hh + 1) * 256, :]
            ).astype(np.float32)  # [256, NQ]
            wqh, wql = _bf16_split(wq, 2)
            blk = np.concatenate([wqh, wql], axis=1)  # [256, 16]
            QW[:, u * 32 : (u + 1) * 32] = (
                blk.reshape(2, 128, 2 * NQ).transpose(1, 0, 2).reshape(128, 32)
            )
        in_maps.append({"AT": AT, "BT": BT, "QW": QW, "QF": QF})
    return in_maps


def _diag_constant():
    """F value the device computes on the (unmasked) pair-grid diagonal.

    d2 on the diagonal is |err| < 2e-4, and F(x) = erf(sqrt((x+B)/2)) /
    sqrt(x+B) is flat there (variation < 1e-4 relative), so every diagonal
    element lands on the same bf16 value: bf16 applied to the device's
    erf * quadratic-reciprocal product at d = sqrt(BIAS). The bf16 bucket
    is 0.4% wide -- vastly wider than the variation -- so this is exact."""
    from scipy.special import erf as _erf

    d0 = np.float32(np.sqrt(BIAS))
    e0 = np.float32(_erf(float(d0) * INV_SQRT2))
    nx = (~d0.reshape(1).view(np.int32)).view(np.float32)[0]
    t = np.float32(d0 * nx)
    rq = np.float32(
        (np.float32(RECIP_A) + t * (np.float32(RECIP_B) + np.float32(RECIP_C) * t))
        * nx
    )
    return float((e0 * rq).astype(BF16))


def kernel(q, r, cell):
    global LAST_RESULTS
    in_maps = _host_prep(q, r)
    nc = _get_program()
    res = run_bass_kernel_spmd(nc, in_maps, list(range(NCORES)), trace=TRACE)
    LAST_RESULTS = res
    S = sum(float(res.results[c]["OUT"][0, 0]) for c in range(NCORES))
    S -= _diag_constant() * float((q.astype(np.float64) ** 2).sum())
    val = S / TWOPI / 2.0 * NORM_FACTOR
    return np.array([val], np.float32)
